# revision 19
# baseline (speedup 1.0000x reference)
"""Trainium2 Bass kernel for nn_MultiHeadedSelfAttention_5179730559275.

Reference math (per batch b):
  q = wq @ x + bq ; k = wk @ x + bk ; v = wv @ x + bv        (1x1 conv, C=256 -> O=256)
  per o-channel (o = head*32 + d), with Q_o,K_o,V_o = 64x64 images [H,W]:
    S_o = Q_o @ K_o^T / sqrt(32); P_o = softmax(S_o, axis=-1); ctx_o = P_o @ V_o

Sharding: data-parallel over batch, 2 batches per core on 8 cores.

Per-core pipeline (per batch):
  1. fp16 projections on PE (lhsT = w^T stationary, rhs = x moving, N=512)
     -> psum [o', 512]; psum->SBUF copies add bias + cast fp16 into
     interleaved layouts pairing o with o+128 (om):
       q16/k16/v16: [j, r, om, c]  (j = o mod 128; r,c image coords)
     V projects from host-transposed x (w-major pixels) so its slices are
     contiguous like q/k.
  2. PE transposes (is_transpose matmul vs fp16 identity) flip 128x128
     slices [j, (om,c)] -> [(om,c), j], 8 per psum bank, then one batched
     copy per bank builds j-major matmul-ready layouts (contiguous per-j
     tiles so PE weight loads/moving fetches run at 1 elem/cycle):
       qS/kS: [om*64 + w, j, h] ; vS: [om*64 + g, j, w] (+ ones col for Z)
  3. Attention per pair j: quadrant matmuls (K=64 at partition bases 0/64):
       S^T psum [om*64+g, h] ; exp (ACT, bias -2) -> eS fp16
       ctx psum [om*64+h, 0:64]=E^T.T@V, col 64 = Z (ones column)
     normalize (bv folded into V bias): obuf = psum * (1/Z); one 16KB-
     descriptor DMA store per om per batch (DRAM layout [b, h, o, w],
     transposed back on host).
"""

import numpy as np

import concourse.bass as bass
import concourse.bacc as bacc
import concourse.tile as tile
from concourse import mybir, masks
from concourse import bass2jax

NCORES = 8
B, C, H, W = 16, 256, 64, 64
O = 256
PIX = H * W
BL = B // NCORES  # batches per core
SCALE = 1.0 / float(np.sqrt(32.0))
EXP_BIAS = -2.0  # softmax-invariant shift keeping exp() well inside fp16 range

FP32 = mybir.dt.float32
FP16 = mybir.dt.float16


def build_kernel(nc: bass.Bass):
    x_in = nc.declare_dram_parameter("x", [BL, C, PIX], FP16, isOutput=False)
    # x with each 64x64 image transposed (w-major pixels); feeds the V
    # projection so V's psum comes out w-major and every transpose input
    # is contiguous.
    xt_in = nc.declare_dram_parameter("xt", [BL, C, PIX], FP16, isOutput=False)
    wT_in = nc.declare_dram_parameter("wT", [3, C, O], FP16, isOutput=False)
    bias_in = nc.declare_dram_parameter("bias", [3, O], FP32, isOutput=False)
    # [b, h, o, w]: one contiguous 16KB run per (h, om) -> single-descriptor
    # DMA rows; host transposes back to [b, o, h, w].
    out = nc.declare_dram_parameter("out", [BL, H, O, W], FP16, isOutput=True)

    with tile.TileContext(nc) as tc:
        with (
            tc.tile_pool(name="singles", bufs=1) as singles,
            tc.tile_pool(name="xin", bufs=2) as xpool,
            tc.tile_pool(name="p16", bufs=1) as p16pool,
            tc.tile_pool(name="tsp", bufs=2) as tpool,
            tc.tile_pool(name="obuf", bufs=1) as opool,
            tc.tile_pool(name="small", bufs=4) as small,
            tc.tile_pool(name="psA", bufs=2, space="PSUM") as psA,
            tc.tile_pool(name="psT", bufs=2, space="PSUM") as psT,
            tc.tile_pool(name="psS", bufs=2, space="PSUM") as psS,
            tc.tile_pool(name="psC", bufs=2, space="PSUM") as psC,
        ):
            # ---- constants loaded once ----
            w_sb = singles.tile([128, 3, 2, O], FP16)  # [c', proj, cc, o]
            nc.sync.dma_start(
                out=w_sb,
                in_=wT_in.rearrange("t (cc c) o -> c t cc o", cc=2),
            )
            bias_sb = singles.tile([128, 3, 2], FP32)  # [o', proj, oc]
            nc.sync.dma_start(
                out=bias_sb,
                in_=bias_in.rearrange("t (oc o) -> o t oc", oc=2),
            )
            expb_sb = singles.tile([128, 1], FP32)
            nc.vector.memset(expb_sb, EXP_BIAS)
            ident = singles.tile([128, 128], FP16)
            masks.make_identity(nc, ident[:])

            tensors = {}
            copy_fns = [
                lambda o_, i_: nc.scalar.copy(o_, i_),
                lambda o_, i_: nc.vector.tensor_copy(o_, i_),
            ]

            def emit_front(b):
                # x pieces ordered by first use (both cc halves of the first
                # pixel range land first); xt goes through the ACT queue so
                # the two DGE engines issue in parallel.
                xsb, xtsb = [], []
                for cc in range(2):
                    xcc = xpool.tile([128, PIX], FP16, tag="xsb")
                    xtcc = xpool.tile([128, PIX], FP16, tag="xtb")
                    xsb.append(xcc)
                    xtsb.append(xtcc)
                NP = 4
                for piece in range(NP):
                    sl = slice(piece * (PIX // NP), (piece + 1) * (PIX // NP))
                    for cc in range(2):
                        nc.sync.dma_start(
                            out=xsb[cc][:, sl],
                            in_=x_in[b, cc * 128 : (cc + 1) * 128, sl],
                        )
                        nc.scalar.dma_start(
                            out=xtsb[cc][:, sl],
                            in_=xt_in[b, cc * 128 : (cc + 1) * 128, sl],
                        )

                # [j, r, om, c]: r,c are image coords; j = o mod 128
                q16 = p16pool.tile([128, H, 2, W], FP16, tag="q16")
                k16 = p16pool.tile([128, H, 2, W], FP16, tag="k16")
                v16 = p16pool.tile([128, H, 2, W], FP16, tag="v16")
                p16 = [q16, k16, v16]

                # nt-outer, proj-inner so consecutive psum drains alternate
                # ACT/DVE and the PE never waits on one engine. (Pool/GpSimd
                # cannot access PSUM on TRN2.)
                for nt in range(8):
                    for proj in range(3):
                        for oc in range(2):
                            ps = psA.tile([128, 512], FP32, tag="ps_proj")
                            xin = xtsb if proj == 2 else xsb
                            for cc in range(2):
                                nc.tensor.matmul(
                                    ps,
                                    lhsT=w_sb[:, proj, cc, oc * 128 : (oc + 1) * 128],
                                    rhs=xin[cc][:, nt * 512 : (nt + 1) * 512],
                                    start=(cc == 0),
                                    stop=(cc == 1),
                                )
                            bias_ap = bias_sb[:, proj, oc : oc + 1]
                            dst = p16[proj][:, nt * 8 : (nt + 1) * 8, oc, :]
                            src = ps.rearrange("p (r c) -> p r c", c=W)
                            if oc == 0:
                                nc.scalar.activation(
                                    out=dst,
                                    in_=src,
                                    func=mybir.ActivationFunctionType.Identity,
                                    bias=bias_ap,
                                    scale=1.0,
                                )
                            else:
                                nc.vector.tensor_scalar_add(
                                    out=dst, in0=src, scalar1=bias_ap
                                )

                # j-major attention layouts: contiguous per-j tiles.
                qS = tpool.tile([128, 128, H], FP16, tag="qS")  # [om*64+w, j, h]
                kS = tpool.tile([128, 128, H], FP16, tag="kS")
                vS = tpool.tile([128, 128, W + 1], FP16, tag="vS")  # [om*64+g, j, w.]
                nc.gpsimd.memset(vS[:, :, W], 1.0)

                # PE transposes: 8 per psum bank, then one batched copy/bank.
                eng_i = 0
                for t16, tS in ((q16, qS), (k16, kS), (v16, vS)):
                    for t0 in range(0, H, 8):
                        pt = psT.tile([128, 8, 128], FP16, tag="pst")
                        for i in range(8):
                            lhsT = t16[:, t0 + i, :, :].rearrange(
                                "p om w -> p (om w)"
                            )
                            nc.tensor.transpose(pt[:, i, :], lhsT, ident)
                        copy_fns[eng_i % 2](tS[:, :, t0 : t0 + 8], pt.rearrange("p i j -> p j i"))
                        eng_i += 1
                tensors[b] = (qS, kS, vS)

            def emit_attn(b):
                qS, kS, vS = tensors[b]
                obuf = opool.tile([128, 128, W], FP16, tag="obuf")  # [om*64+h, j, w]
                JG = 8
                PG = 4
                NJG = 128 // JG

                def emit_S(jg):
                    sp8f = psS.tile([128, 512], FP32, tag="sp8")
                    sp8 = sp8f.rearrange("p (i h) -> p i h", h=H)
                    for i in range(JG):
                        j = jg * JG + i
                        for om in range(2):
                            pr = slice(om * 64, om * 64 + 64)
                            nc.tensor.matmul(
                                sp8[pr, i, :],
                                lhsT=kS[pr, j, :],
                                rhs=qS[pr, j, :],
                                start=True,
                                stop=True,
                            )
                    eS8 = small.tile([128, JG, H], FP16, tag="eS8")
                    nc.scalar.activation(
                        out=eS8,
                        in_=sp8,
                        func=mybir.ActivationFunctionType.Exp,
                        bias=expb_sb,
                        scale=1.0,
                    )
                    return eS8

                def emit_ctx(jg, eS8):
                    for sg in range(2):
                        j0 = jg * JG + sg * PG
                        cp4f = psC.tile([128, 512], FP32, tag="cp4")
                        cp4 = cp4f[:, 0 : PG * (W + 1)].rearrange(
                            "p (i c) -> p i c", c=W + 1
                        )
                        for i in range(PG):
                            j = j0 + i
                            for om in range(2):
                                pr = slice(om * 64, om * 64 + 64)
                                nc.tensor.matmul(
                                    cp4[pr, i, :],
                                    lhsT=eS8[pr, j - jg * JG, :],
                                    rhs=vS[pr, j, :],
                                    start=True,
                                    stop=True,
                                )
                        rz4 = small.tile([128, PG], FP32, tag="rz4")
                        nc.vector.reciprocal(out=rz4, in_=cp4[:, :, W])
                        # bv folded into the V projection bias -> normalize
                        # is one broadcast multiply per group.
                        rzf = rz4[:]
                        rzb = bass.AP(
                            tensor=rzf.tensor,
                            offset=rzf.offset,
                            ap=[rzf.ap[0], rzf.ap[1], [0, W]],
                        )
                        nc.vector.tensor_mul(
                            out=obuf[:, j0 : j0 + PG, :],
                            in0=cp4[:, :, 0:W],
                            in1=rzb,
                        )

                # software pipeline: ctx(jg) runs two S-groups later so the
                # exp() round-trip never stalls the PE.
                pend = []
                for jg in range(NJG):
                    pend.append((jg, emit_S(jg)))
                    if len(pend) == 2:
                        g, e = pend.pop(0)
                        emit_ctx(g, e)
                for g, e in pend:
                    emit_ctx(g, e)

                for om in range(2):
                    nc.sync.dma_start(
                        out=out[b, :, om * 128 : (om + 1) * 128, :],
                        in_=obuf[om * 64 : om * 64 + 64, :, :],
                    )

            for b in range(BL):
                emit_front(b)
            for b in range(BL):
                emit_attn(b)
    return nc


_NC_CACHE = {}


def get_nc():
    if "nc" not in _NC_CACHE:
        nc = bacc.Bacc(None, target_bir_lowering=False)
        build_kernel(nc)
        nc.finalize()
        _NC_CACHE["nc"] = nc
    return _NC_CACHE["nc"]


def prep_in_maps(x, wq, bq, wk, bk, wv, bv):
    wT = np.stack(
        [
            np.ascontiguousarray((wq * SCALE).T),
            np.ascontiguousarray(wk.T),
            np.ascontiguousarray(wv.T),
        ]
    ).astype(np.float16)
    # bv is folded into the V projection bias: softmax weights sum to 1, so
    # (sum_g P*(V+bv)) == (sum_g P*V) + bv exactly.
    biases = np.stack([bq * SCALE, bk, bv]).astype(np.float32)
    x16 = x.astype(np.float16)
    xs = np.ascontiguousarray(x16.reshape(NCORES, BL, C, PIX))
    xts = np.ascontiguousarray(
        x16.reshape(NCORES, BL, C, H, W).transpose(0, 1, 2, 4, 3)
    ).reshape(NCORES, BL, C, PIX)
    return [
        {"x": xs[i], "xt": xts[i], "wT": wT, "bias": biases} for i in range(NCORES)
    ]


def gather_outs(results):
    # device out is [BL, H, O, W]; transpose back to [BL, O, H, W]
    outs = [
        np.asarray(r["out"]).reshape(BL, H, O, W).transpose(0, 2, 1, 3)
        for r in results
    ]
    return np.concatenate(outs, axis=0).astype(np.float32)


def kernel(x, wq, bq, wk, bk, wv, bv):
    nc = get_nc()
    in_maps = prep_in_maps(x, wq, bq, wk, bk, wv, bv)
    results = bass2jax.run_bass_via_pjrt(nc, in_maps, n_cores=NCORES)
    return gather_outs(results)
